# revision 50
# baseline (speedup 1.0000x reference)
"""GCNEncoder (GCNConv + TransformerEncoderLayer) on 8 Trainium2 NeuronCores.

v2 design (vs baseline): the normalized dense adjacency A (with self loops
and symmetric degree normalization baked in) is built on the HOST and DMAed
as fp16, eliminating the GPSIMD scatter path, duplicate-edge merge matmuls
and on-device degree computation.  Nodes are split 512/core.  Per core:
  - GCN: xw = x @ W_gcn computed replicated (j-chunk pipelined behind its
    own DMA), aggregation = dense fp16 matmul A^T[4096 src, 512 dst] @ xw,
    with the aggregation k-chunks pipelined behind the xw chunks.
  - Attention: q/k quantized to fp8e4m3 (scores are O(0.2) for this model
    family so the softmax is insensitive to fp8 noise; validated offline).
    One packed AllGather moves K (fp8) + V (fp16) as int32 words; the
    collective has a large fixed cost so a single small op beats two.
    exp(s) is fp16 (fp8 activation writes run ~1.5x slower); softmax
    denominators accumulate on the vector engine and reduce via a
    ones-vector matmul.
  - K-bias dropped: it multiplies each query row's exp() by a constant
    factor which cancels exactly in the softmax normalization.
  - V-bias folded into the out_proj bias on host (softmax rows sum to 1).
  - LayerNorm normalization runs on the scalar engine (per-node stats are
    per-partition scalars); residual biases pre-added off the critical path.
All big matmul operands fp16 (GCN/FFN/PV; fp8 there breaks the 2e-2 gate
or is no faster), accumulation fp32 in PSUM.
"""

import math

import numpy as np

import concourse.bacc as bacc
import concourse.mybir as mybir
import concourse.tile as tile

N_CORES = 8
N = 4096
DIN = 512
D = 256
H = 2
DH = 128
DFF = 2048
EPS = 1e-5
P = 128

NPC = N // N_CORES          # nodes per core = 512
MPC = NPC // P              # m-chunks per core = 4
JT = N // P                 # node chunks of full graph = 32
KD = DIN // P               # din k-tiles = 4
DC = DFF // P               # dff chunks = 16

DT32 = mybir.dt.float32
DT16 = mybir.dt.float16
DT8 = mybir.dt.float8e4
F = mybir.ActivationFunctionType
A = mybir.AluOpType
DR = mybir.MatmulPerfMode.DoubleRow
INV_SQRT_DH = 1.0 / math.sqrt(DH)


def build_kernel():
    nc = bacc.Bacc("TRN2", target_bir_lowering=False, debug=False,
                   num_devices=N_CORES)

    def din(name, shape, dt=DT32):
        return nc.dram_tensor(name, shape, dt, kind="ExternalInput")

    xj_d = din("xj", [P, JT * DIN], DT16)     # x.T wrapped j-major
    ag_d = din("ag", [P, JT * NPC], DT16)     # A^T slice, kt-major
    wg_d = din("wg", [P, KD * D], DT16)
    winT_d = din("winT", [P, 6 * D], DT16)    # (q,k,v)x(h0,h1) blocks
    woT_d = din("woT", [P, H * D], DT16)
    w1T_d = din("w1T", [P, 2 * DFF], DT16)
    w2T_d = din("w2T", [P, DC * D], DT16)
    bias_d = din("bias", [P, 7 * D])          # host-broadcast rows
    b1t_d = din("b1t", [P, DC])
    qb_d = din("qb", [P, H])
    ident_d = din("ident", [P, P], DT16)

    out_d = nc.dram_tensor("out", [NPC, D], DT32, kind="ExternalOutput")

    with tile.TileContext(nc) as tc:
        with (
            tc.tile_pool(name="keep", bufs=1) as keep,
            tc.tile_pool(name="dram", bufs=1, space="DRAM") as dram,
        ):
            ctx_gcn = tc.tile_pool(name="gcn_keep", bufs=1)
            gk = ctx_gcn.__enter__()

            # ---- input DMAs, ordered by first use ----
            wg16 = gk.tile([P, KD * D], DT16)
            nc.sync.dma_start(wg16[:], wg_d[:])
            xj16 = gk.tile([P, JT * DIN], DT16)
            ag16 = gk.tile([P, JT * NPC], DT16)
            # tiny first chunk so xw(0) starts as early as possible
            nc.sync.dma_start(xj16[:, 0:DIN], xj_d[:, 0:DIN])
            nc.sync.dma_start(xj16[:, DIN:4 * DIN], xj_d[:, DIN:4 * DIN])
            nc.sync.dma_start(ag16[:, 0:4 * NPC], ag_d[:, 0:4 * NPC])
            NCH = 7
            for c in range(NCH):
                w = 4 * DIN
                nc.sync.dma_start(xj16[:, (c + 1) * w:(c + 2) * w],
                                  xj_d[:, (c + 1) * w:(c + 2) * w])
                w2 = 4 * NPC
                nc.sync.dma_start(ag16[:, (c + 1) * w2:(c + 2) * w2],
                                  ag_d[:, (c + 1) * w2:(c + 2) * w2])
            bias_bc = keep.tile([P, 7 * D], DT32)
            nc.gpsimd.dma_start(bias_bc[:], bias_d[:])
            ident16 = keep.tile([P, P], DT16)
            nc.gpsimd.dma_start(ident16[:], ident_d[:])
            winT16 = keep.tile([P, 6 * D], DT16)
            nc.gpsimd.dma_start(winT16[:], winT_d[:])
            qb = keep.tile([P, H], DT32)
            nc.gpsimd.dma_start(qb[:], qb_d[:])
            woT16 = keep.tile([P, H * D], DT16)
            nc.gpsimd.dma_start(woT16[:], woT_d[:])
            b1t = keep.tile([P, DC], DT32)
            nc.gpsimd.dma_start(b1t[:], b1t_d[:])

            bgcn_bc = bias_bc[:, 0:D]
            b2_bc = bias_bc[:, D:2 * D]
            ln1g_bc = bias_bc[:, 2 * D:3 * D]
            ln1b_bc = bias_bc[:, 3 * D:4 * D]
            ln2g_bc = bias_bc[:, 4 * D:5 * D]
            ln2b_bc = bias_bc[:, 5 * D:6 * D]
            bo_bc = bias_bc[:, 6 * D:7 * D]

            ones16 = keep.tile([P, 1], DT16)
            nc.vector.memset(ones16[:], 1.0)
            ones32 = keep.tile([1, 1], DT32)
            nc.vector.memset(ones32[:], 1.0)

            def bc4(ap_2d):
                return ap_2d[:, None, :].to_broadcast([P, MPC, D])

            def r3(ap_2d, t):
                """[P, t*n] -> [P, t, n] view"""
                return ap_2d.rearrange("p (t n) -> p t n", t=t)

            # ================= GCN =================
            xws16 = gk.tile([P, JT * D], DT16)
            h16 = keep.tile([P, MPC * D], DT16)
            h16b = keep.tile([P, MPC * D], DT16)
            hT16 = keep.tile([P, 2 * NPC], DT16)

            with tc.tile_pool(name="xw_ps", bufs=4, space="PSUM") as xps, \
                 tc.tile_pool(name="agg_ps", bufs=1, space="PSUM") as aps, \
                 tc.tile_pool(name="gcn_sb", bufs=2) as gsb:
                agg_ps = [aps.tile([P, D], DT32, space="PSUM",
                                   tag=f"agg{m}", name=f"agg{m}")
                          for m in range(MPC)]

                def emit_xw(j):
                    pxw = xps.tile([P, D], DT32, space="PSUM", tag="xw")
                    for k in range(KD):
                        nc.tensor.matmul(
                            pxw[:],
                            lhsT=xj16[:, DIN * j + P * k:DIN * j + P * (k + 1)],
                            rhs=wg16[:, D * k:D * (k + 1)],
                            start=(k == 0), stop=(k == KD - 1))
                    eng = nc.vector if j % 2 == 0 else nc.scalar
                    if j % 2 == 0:
                        eng.tensor_copy(xws16[:, D * j:D * (j + 1)], pxw[:])
                    else:
                        eng.copy(xws16[:, D * j:D * (j + 1)], pxw[:])

                def emit_agg(j):
                    for m in range(MPC):
                        nc.tensor.matmul(
                            agg_ps[m][:],
                            lhsT=ag16[:, NPC * j + P * m:NPC * j + P * (m + 1)],
                            rhs=xws16[:, D * j:D * (j + 1)],
                            start=(j == 0), stop=(j == JT - 1))

                emit_xw(0)
                emit_xw(1)
                for j in range(2, JT):
                    emit_xw(j)
                    emit_agg(j - 2)
                emit_agg(JT - 2)
                emit_agg(JT - 1)

                # h = relu(agg + b_gcn)
                x_all = gsb.tile([P, MPC * D], DT32, tag="xall")
                for m in range(MPC):
                    nc.vector.tensor_tensor(
                        x_all[:, D * m:D * (m + 1)], agg_ps[m][:],
                        bgcn_bc, op=A.add)
                nc.scalar.activation(h16[:], x_all[:], F.Relu)
                # h + out_proj bias, pre-added for the LN1 residual
                nc.vector.tensor_tensor(
                    h16b[:].rearrange("p (m d) -> p m d", m=MPC),
                    h16[:].rearrange("p (m d) -> p m d", m=MPC),
                    bc4(bo_bc), op=A.add)

            # transpose h -> hT16 (feature-major) via matmul with identity;
            # 4 transposes per psum tile, one batched copy each
            with tc.tile_pool(name="tr_ps", bufs=2, space="PSUM") as tps:
                for f in range(2):
                    ptr = tps.tile([P, NPC], DT32, space="PSUM", tag="tr")
                    for m in range(MPC):
                        nc.tensor.matmul(
                            ptr[:, P * m:P * (m + 1)],
                            lhsT=h16[:, D * m + P * f:D * m + P * (f + 1)],
                            rhs=ident16[:], start=True, stop=True)
                    dst = hT16[:, NPC * f:NPC * (f + 1)]
                    if f == 0:
                        nc.vector.tensor_copy(dst, ptr[:])
                    else:
                        nc.scalar.copy(dst, ptr[:])

            ctx_gcn.__exit__(None, None, None)

            # ================= QKV (q/k fp8: scores are insensitive) =====
            qT8 = keep.tile([P, H * NPC], DT8)
            kT8 = keep.tile([P, H * NPC], DT8)
            v16 = keep.tile([P, H * NPC], DT16)
            DTI32E = mybir.dt.int32
            # per-head bounce [K | V-lo | V-hi] as int32 row-blocks
            kvb = [dram.tile([3 * P, P], DTI32E, name=f"kvb{h}")
                   for h in range(H)]
            kvg = [dram.tile([N_CORES * 3 * P, P], DTI32E,
                             addr_space="Shared", name=f"kvg{h}")
                   for h in range(H)]
            with tc.tile_pool(name="kv_ps", bufs=3, space="PSUM") as kvps:
                for h in range(H):
                    pq = kvps.tile([P, NPC], DT32, space="PSUM", tag="kv")
                    for k in range(2):
                        nc.tensor.matmul(
                            pq[:],
                            lhsT=winT16[:, D * h + P * k:D * h + P * (k + 1)],
                            rhs=hT16[:, NPC * k:NPC * (k + 1)],
                            start=(k == 0), stop=(k == 1))
                    nc.vector.tensor_scalar(qT8[:, NPC * h:NPC * (h + 1)],
                                            pq[:], qb[:, h:h + 1], None,
                                            op0=A.add)
                    pk = kvps.tile([P, NPC], DT32, space="PSUM", tag="kv")
                    for k in range(2):
                        nc.tensor.matmul(
                            pk[:],
                            lhsT=winT16[:, D * (2 + h) + P * k:
                                        D * (2 + h) + P * (k + 1)],
                            rhs=hT16[:, NPC * k:NPC * (k + 1)],
                            start=(k == 0), stop=(k == 1))
                    # k bias dropped: cancels in softmax normalization
                    nc.vector.tensor_copy(kT8[:, NPC * h:NPC * (h + 1)], pk[:])
                # V per head; bounce + gather each head as soon as ready so
                # head-1's collective overlaps head-0's loads/scores/PV
                for h in range(H):
                    for m in range(MPC):
                        pv = kvps.tile([P, P], DT32, space="PSUM", tag="kvv")
                        for k in range(2):
                            nc.tensor.matmul(
                                pv[:],
                                lhsT=hT16[:, NPC * k + P * m:NPC * k + P * (m + 1)],
                                rhs=winT16[:, D * (4 + h) + P * k:
                                            D * (4 + h) + P * (k + 1)],
                                start=(k == 0), stop=(k == 1))
                        dst = v16[:, NPC * h + P * m:NPC * h + P * (m + 1)]
                        if m % 2 == 0:
                            nc.vector.tensor_copy(dst, pv[:])
                        else:
                            nc.scalar.copy(dst, pv[:])
                    kvb_v = kvb[h][:].rearrange("(x p) n -> p x n", p=P)
                    nc.sync.dma_start(
                        kvb_v[:, 0, :],
                        kT8[:, NPC * h:NPC * (h + 1)].bitcast(DTI32E))
                    nc.sync.dma_start(
                        kvb_v[:, 1:3, :],
                        v16[:, NPC * h:NPC * (h + 1)].bitcast(DTI32E)
                        .rearrange("p (x n) -> p x n", x=2))
                    nc.gpsimd.collective_compute(
                        "AllGather", A.bypass,
                        replica_groups=[list(range(N_CORES))],
                        ins=[kvb[h].opt()], outs=[kvg[h].opt()])

            DTI32 = DTI32E
            # FFN weights stream while the gathers run
            w1T16 = keep.tile([P, 2 * DFF], DT16)
            nc.sync.dma_start(w1T16[:], w1T_d[:])
            w2T16 = keep.tile([P, DC * D], DT16)
            nc.sync.dma_start(w2T16[:], w2T_d[:])

            kT8f = keep.tile([P, H * N], DT8)
            v16f = keep.tile([P, H * N], DT16)
            for h, eng in ((0, nc.scalar), (1, nc.gpsimd)):
                gvh = kvg[h][:].rearrange("(g x p) n -> x p g n",
                                          g=N_CORES, x=3, p=P)
                eng.dma_start(
                    kT8f[:, N * h:N * (h + 1)].bitcast(DTI32).rearrange(
                        "p (g n) -> p g n", g=N_CORES), gvh[0])
                v16f_v = v16f[:, N * h:N * (h + 1)].bitcast(DTI32).rearrange(
                    "p (g x n) -> p g x n", g=N_CORES, x=2, n=P)
                for x in range(2):
                    eng.dma_start(v16f_v[:, :, x, :], gvh[1 + x])

            # ================= attention =================
            oT16 = keep.tile([P, H * NPC], DT16)
            recT = keep.tile([P, H * MPC], DT32)
            KT2 = JT // 2
            with tc.tile_pool(name="att_sb", bufs=4) as atsb, \
                 tc.tile_pool(name="att_ps", bufs=1, space="PSUM") as atps, \
                 tc.tile_pool(name="s_ps", bufs=2, space="PSUM") as sps:
                o_ps = [atps.tile([P, NPC], DT32, space="PSUM",
                                  tag=f"o{h}", name=f"o{h}")
                        for h in range(H)]
                # shared bank region: softmax sums in row 0 of cols 0:512,
                # transposed sums in cols 512:516 (all partitions)
                fin = atps.tile([P, 2 * NPC], DT32, space="PSUM",
                                tag="fin", name="fin")
                esum = [None, None]
                pend = []

                def head_fin(h):
                    """denominator -> per-partition reciprocals; overlaps
                    the other head's scores/PV stream"""
                    nc.vector.tensor_copy(oT16[:, NPC * h:NPC * (h + 1)],
                                          o_ps[h][:])
                    for u in range(2):
                        nc.tensor.matmul(
                            fin[0:1, 0:NPC], lhsT=ones16[:],
                            rhs=esum[h][:, NPC * u:NPC * (u + 1)],
                            start=(u == 0), stop=(u == 1))
                    srow = atsb.tile([1, NPC], DT32, tag="srow")
                    nc.vector.tensor_copy(srow[:], fin[0:1, 0:NPC])
                    for m in range(MPC):
                        nc.tensor.transpose(
                            fin[:, NPC + m:NPC + m + 1],
                            srow[:, P * m:P * (m + 1)], ones32[:])
                    nc.vector.reciprocal(recT[:, MPC * h:MPC * (h + 1)],
                                         fin[:, NPC:NPC + MPC])

                def emit_pv(h, kt2, es16):
                    for u in range(2):
                        kt = 2 * kt2 + u
                        nc.tensor.matmul(
                            o_ps[h][:],
                            lhsT=v16f[:, N * h + P * kt:N * h + P * (kt + 1)],
                            rhs=es16[:, NPC * u:NPC * (u + 1)],
                            start=(kt == 0), stop=(kt == JT - 1))
                    if kt2 == 0:
                        nc.vector.tensor_copy(esum[h][:], es16[:])
                    else:
                        nc.vector.tensor_add(esum[h][:], esum[h][:], es16[:])
                    if kt2 == KT2 - 1:
                        head_fin(h)

                for h in range(H):
                    esum[h] = atsb.tile([P, 2 * NPC], DT16,
                                        tag=f"eac{h}", name=f"eacc{h}")
                    for kt2 in range(KT2):
                        ps_s = sps.tile([P, 2 * NPC], DT32, space="PSUM",
                                        tag="S")
                        for u in range(2):
                            kt = 2 * kt2 + u
                            nc.tensor.matmul(
                                ps_s[:, NPC * u:NPC * (u + 1)],
                                lhsT=kT8f[:, N * h + P * kt:N * h + P * (kt + 1)],
                                rhs=qT8[:, NPC * h:NPC * (h + 1)],
                                start=True, stop=True)
                        es16 = atsb.tile([P, 2 * NPC], DT16, tag="es")
                        nc.scalar.activation(es16[:], ps_s[:], F.Exp,
                                             scale=INV_SQRT_DH)
                        pend.append((h, kt2, es16))
                        if len(pend) > 2:
                            emit_pv(*pend.pop(0))
                for args in pend:
                    emit_pv(*args)

            # ============ out_proj + residual + LN1 ============
            h1_16 = keep.tile([P, MPC * D], DT16)
            h1T16 = keep.tile([P, 2 * NPC], DT16)
            with tc.tile_pool(name="ln_sb", bufs=2) as lsb, \
                 tc.tile_pool(name="op_ps", bufs=4, space="PSUM") as ops:

                def layernorm_all(dst, x_all, g_sl, b_sl, tag):
                    mu4 = lsb.tile([P, MPC], DT32, tag=f"{tag}mu")
                    nc.vector.tensor_reduce(
                        mu4[:], x_all[:].rearrange("p (m d) -> p m d", m=MPC),
                        axis=mybir.AxisListType.X, op=A.add)
                    negmu4 = lsb.tile([P, MPC], DT32, tag=f"{tag}nm")
                    nc.vector.tensor_scalar(negmu4[:], mu4[:], -1.0 / D, None,
                                            op0=A.mult)
                    sq4 = lsb.tile([P, D], DT32, tag=f"{tag}sq")
                    ssq4 = lsb.tile([P, MPC], DT32, tag=f"{tag}ss")
                    for m in range(MPC):
                        nc.scalar.activation(sq4[:], x_all[:, D * m:D * (m + 1)],
                                             F.Square, bias=negmu4[:, m:m + 1],
                                             accum_out=ssq4[:, m:m + 1])
                    var4 = lsb.tile([P, MPC], DT32, tag=f"{tag}vr")
                    nc.vector.tensor_scalar(var4[:], ssq4[:], 1.0 / D, EPS,
                                            op0=A.mult, op1=A.add)
                    sd4 = lsb.tile([P, MPC], DT32, tag=f"{tag}sd")
                    nc.scalar.activation(sd4[:], var4[:], F.Sqrt)
                    rstd4 = lsb.tile([P, MPC], DT32, tag=f"{tag}rs")
                    nc.vector.reciprocal(rstd4[:], sd4[:])
                    nmr4 = lsb.tile([P, MPC], DT32, tag=f"{tag}nr")
                    nc.vector.tensor_mul(nmr4[:], negmu4[:], rstd4[:])
                    # normalize split across scalar and vector engines
                    # (per-node stats are per-partition scalars)
                    xc = lsb.tile([P, MPC * D], DT32, tag=f"{tag}xc")
                    for m in range(MPC):
                        if m % 2 == 0:
                            nc.scalar.activation(
                                xc[:, D * m:D * (m + 1)],
                                x_all[:, D * m:D * (m + 1)],
                                F.Identity, bias=nmr4[:, m:m + 1],
                                scale=rstd4[:, m:m + 1])
                        else:
                            nc.vector.tensor_scalar(
                                xc[:, D * m:D * (m + 1)],
                                x_all[:, D * m:D * (m + 1)],
                                negmu4[:, m:m + 1], rstd4[:, m:m + 1],
                                op0=A.add, op1=A.mult)
                    nc.vector.tensor_tensor(
                        xc[:].rearrange("p (m d) -> p m d", m=MPC),
                        xc[:].rearrange("p (m d) -> p m d", m=MPC),
                        bc4(g_sl), op=A.mult)
                    nc.vector.tensor_tensor(
                        dst[:].rearrange("p (m d) -> p m d", m=MPC),
                        xc[:].rearrange("p (m d) -> p m d", m=MPC),
                        bc4(b_sl), op=A.add)

                x1_all = lsb.tile([P, MPC * D], DT32, tag="x1all")
                for m in range(MPC):
                    pa = [None, None]
                    for h in range(H):
                        pa[h] = ops.tile([P, D], DT32, space="PSUM", tag="op",
                                         name=f"pa{h}")
                        nc.tensor.matmul(
                            pa[h][:],
                            lhsT=oT16[:, NPC * h + P * m:NPC * h + P * (m + 1)],
                            rhs=woT16[:, D * h:D * (h + 1)],
                            start=True, stop=True)
                    t0m = lsb.tile([P, D], DT32, tag="t0m")
                    nc.scalar.activation(t0m[:], pa[0][:], F.Copy,
                                         scale=recT[:, m:m + 1])
                    nc.vector.scalar_tensor_tensor(
                        t0m[:], pa[1][:], recT[:, MPC + m:MPC + m + 1],
                        t0m[:], op0=A.mult, op1=A.add)
                    nc.vector.tensor_tensor(x1_all[:, D * m:D * (m + 1)],
                                            t0m[:], h16b[:, D * m:D * (m + 1)],
                                            op=A.add)
                layernorm_all(h1_16, x1_all, ln1g_bc, ln1b_bc, "a")
                # h1 + lin2 bias, pre-added for the LN2 residual (runs
                # while FFN matmuls stream)
                h1b = lsb.tile([P, MPC * D], DT16, tag="h1b")
                nc.vector.tensor_tensor(
                    h1b[:].rearrange("p (m d) -> p m d", m=MPC),
                    h1_16[:].rearrange("p (m d) -> p m d", m=MPC),
                    bc4(b2_bc), op=A.add)

                with tc.tile_pool(name="tr2_ps", bufs=2, space="PSUM") as tps2:
                    for f in range(2):
                        ptr = tps2.tile([P, NPC], DT32, space="PSUM",
                                        tag="tr2")
                        for m in range(MPC):
                            nc.tensor.matmul(
                                ptr[:, P * m:P * (m + 1)],
                                lhsT=h1_16[:, D * m + P * f:D * m + P * (f + 1)],
                                rhs=ident16[:], start=True, stop=True)
                        dst = h1T16[:, NPC * f:NPC * (f + 1)]
                        if f == 0:
                            nc.vector.tensor_copy(dst, ptr[:])
                        else:
                            nc.scalar.copy(dst, ptr[:])

                # ================= FFN =================
                out_sb = keep.tile([P, MPC * D], DT32)
                ff1T16 = keep.tile([P, DC * NPC], DT16)
                with tc.tile_pool(name="f1_ps", bufs=3, space="PSUM") as fps:
                    for dc in range(DC):
                        pf = fps.tile([P, NPC], DT32, space="PSUM", tag="f1")
                        for k in range(2):
                            nc.tensor.matmul(
                                pf[:],
                                lhsT=w1T16[:, DFF * k + P * dc:
                                           DFF * k + P * (dc + 1)],
                                rhs=h1T16[:, NPC * k:NPC * (k + 1)],
                                start=(k == 0), stop=(k == 1))
                        nc.scalar.activation(
                            ff1T16[:, NPC * dc:NPC * (dc + 1)], pf[:], F.Relu,
                            bias=b1t[:, dc:dc + 1])

                x2_all = lsb.tile([P, MPC * D], DT32, tag="x2all")
                with tc.tile_pool(name="f2_ps", bufs=2, space="PSUM") as fps2:
                    for m in range(MPC):
                        pf2 = fps2.tile([P, D], DT32, space="PSUM", tag="f2")
                        for dc in range(DC):
                            nc.tensor.matmul(
                                pf2[:],
                                lhsT=ff1T16[:, NPC * dc + P * m:
                                            NPC * dc + P * (m + 1)],
                                rhs=w2T16[:, D * dc:D * (dc + 1)],
                                start=(dc == 0), stop=(dc == DC - 1))
                        nc.vector.tensor_tensor(
                            x2_all[:, D * m:D * (m + 1)], pf2[:],
                            h1b[:, D * m:D * (m + 1)], op=A.add)
                layernorm_all(out_sb, x2_all, ln2g_bc, ln2b_bc, "b")
                nc.scalar.dma_start(
                    out_d[:].rearrange("(m p) d -> p m d", p=P),
                    out_sb[:].rearrange("p (m d) -> p m d", m=MPC))

    nc.compile()
    return nc


# ======================= host-side prep =======================

def _prep_inputs(x, edge_index, edge_weight, W_gcn, b_gcn, in_proj_w,
                 in_proj_b, out_proj_w, out_proj_b, lin1_w, lin1_b, lin2_w,
                 lin2_b, ln1_g, ln1_b, ln2_g, ln2_b):
    """Layout prep + dense normalized-adjacency build. Per-core input maps."""
    f16 = np.float16
    x = np.asarray(x, np.float32)
    src = np.asarray(edge_index[0], np.int64)
    dst = np.asarray(edge_index[1], np.int64)
    w = np.asarray(edge_weight, np.float32)

    # dense A with self loops + symmetric degree norm (matches PyG GCNConv)
    loops = np.arange(N)
    s_all = np.concatenate([src, loops])
    d_all = np.concatenate([dst, loops])
    w_all = np.concatenate([w, np.ones(N, np.float32)])
    deg = np.bincount(d_all, weights=w_all, minlength=N).astype(np.float32)
    dinv = np.where(deg > 0, 1.0 / np.sqrt(deg), 0.0).astype(np.float32)
    norm = dinv[s_all] * w_all * dinv[d_all]
    Afull = np.bincount(s_all * N + d_all, weights=norm,
                        minlength=N * N).reshape(N, N).astype(f16)

    # xj[p, j*512 + k*128 + m] = x[128j+m, 128k+p]
    xj = np.ascontiguousarray(
        x.reshape(JT, P, KD, P).transpose(3, 0, 2, 1).reshape(P, -1)).astype(f16)
    # wg[p, k*256+f] = W_gcn[128k+p, f]
    wg = np.ascontiguousarray(
        np.asarray(W_gcn, np.float32).reshape(KD, P, D).transpose(1, 0, 2)
        .reshape(P, -1)).astype(f16)
    # winT[p, (2*sec+h)*256 + k*128 + m] = in_proj_w[sec*256 + h*128 + m, 128k+p]
    winT = np.ascontiguousarray(
        np.asarray(in_proj_w, np.float32).reshape(3, H, P, 2, P)
        .transpose(4, 0, 1, 3, 2).reshape(P, -1)).astype(f16)
    # woT[p, h*256+f] = out_proj_w[f, h*128+p]
    woT = np.ascontiguousarray(
        np.asarray(out_proj_w, np.float32).reshape(D, H, P)
        .transpose(2, 1, 0).reshape(P, -1)).astype(f16)
    # w1T[p, k*2048 + dc*128 + m] = lin1_w[128dc+m, 128k+p]
    w1T = np.ascontiguousarray(
        np.asarray(lin1_w, np.float32).reshape(DC, P, 2, P)
        .transpose(3, 2, 0, 1).reshape(P, -1)).astype(f16)
    # w2T[p, dc*256+f] = lin2_w[f, 128dc+p]
    w2T = np.ascontiguousarray(
        np.asarray(lin2_w, np.float32).reshape(D, DC, P)
        .transpose(2, 1, 0).reshape(P, -1)).astype(f16)

    # V bias folded into out_proj bias (softmax rows sum to 1)
    vb = np.asarray(in_proj_b, np.float32)[2 * D:3 * D]
    bo_eff = np.asarray(out_proj_b, np.float32) + \
        vb @ np.asarray(out_proj_w, np.float32).T
    bias_rows = np.concatenate([
        np.asarray(v, np.float32).reshape(-1) for v in
        (b_gcn, lin2_b, ln1_g, ln1_b, ln2_g, ln2_b, bo_eff)])
    bias = np.tile(bias_rows.reshape(1, -1), (P, 1)).astype(np.float32)
    b1t = np.ascontiguousarray(
        np.asarray(lin1_b, np.float32).reshape(DC, P).T)
    qb = np.ascontiguousarray(
        np.asarray(in_proj_b, np.float32)[0:D].reshape(H, P).T)
    ident = np.eye(P, dtype=f16)

    shared = {"xj": xj, "wg": wg, "winT": winT, "woT": woT, "w1T": w1T,
              "w2T": w2T, "bias": bias, "b1t": b1t, "qb": qb, "ident": ident}
    in_maps = []
    for c in range(N_CORES):
        # ag[p, kt*512 + n] = A[128kt+p, 512c+n]
        ag = np.ascontiguousarray(
            Afull[:, NPC * c:NPC * (c + 1)].reshape(JT, P, NPC)
            .transpose(1, 0, 2).reshape(P, -1))
        in_maps.append({**shared, "ag": ag})
    return in_maps


# ======================= runner =======================

class _Runner:
    """Persistent-jit SPMD executor (mirrors bass2jax.run_bass_via_pjrt)."""

    def __init__(self, nc):
        import jax
        from jax.sharding import Mesh, PartitionSpec
        from jax.experimental.shard_map import shard_map
        from concourse.bass2jax import (_bass_exec_p, install_neuronx_cc_hook,
                                        partition_id_tensor)
        install_neuronx_cc_hook()
        self.jax = jax
        partition_name = (nc.partition_id_tensor.name
                          if nc.partition_id_tensor else None)
        in_names, out_names, out_avals, zero_outs = [], [], [], []
        for alloc in nc.m.functions[0].allocations:
            if not isinstance(alloc, mybir.MemoryLocationSet):
                continue
            name = alloc.memorylocations[0].name
            if alloc.kind == "ExternalInput":
                if name != partition_name:
                    in_names.append(name)
            elif alloc.kind == "ExternalOutput":
                out_names.append(name)
                shape = tuple(alloc.tensor_shape)
                dtype = mybir.dt.np(alloc.dtype)
                out_avals.append(jax.core.ShapedArray(shape, dtype))
                zero_outs.append(np.zeros(shape, dtype))
        self.in_names, self.out_names = in_names, out_names
        self.out_shapes = [tuple(a.shape) for a in out_avals]
        self.n_params = len(in_names)
        self.zero_outs = zero_outs
        all_in = in_names + out_names
        if partition_name is not None:
            all_in.append(partition_name)

        def _body(*args):
            operands = list(args)
            if partition_name is not None:
                operands.append(partition_id_tensor())
            return tuple(_bass_exec_p.bind(
                *operands, out_avals=tuple(out_avals), in_names=tuple(all_in),
                out_names=tuple(out_names), lowering_input_output_aliases=(),
                sim_require_finite=True, sim_require_nnan=True, nc=nc))

        devices = jax.devices()[:N_CORES]
        self.mesh = Mesh(np.asarray(devices), ("core",))
        nin = self.n_params + len(out_names)
        self.fn = jax.jit(
            shard_map(_body, mesh=self.mesh,
                      in_specs=(PartitionSpec("core"),) * nin,
                      out_specs=(PartitionSpec("core"),) * len(out_names),
                      check_rep=False),
            keep_unused=True)

    def place(self, in_maps):
        import jax
        from jax.sharding import PartitionSpec
        per_core = [[np.asarray(m[n]) for n in self.in_names] for m in in_maps]
        concat = [np.concatenate([per_core[c][i] for c in range(N_CORES)], axis=0)
                  for i in range(self.n_params)]
        zeros = [np.zeros((N_CORES * z.shape[0], *z.shape[1:]), z.dtype)
                 for z in self.zero_outs]
        sh = jax.sharding.NamedSharding(self.mesh, PartitionSpec("core"))
        return [jax.device_put(a, sh) for a in (*concat, *zeros)]

    def run(self, args):
        outs = self.fn(*args)
        self.jax.block_until_ready(outs)
        return outs

    def results(self, outs):
        res = []
        for c in range(N_CORES):
            d = {}
            for i, name in enumerate(self.out_names):
                full = np.asarray(outs[i])
                ps = self.out_shapes[i]
                d[name] = full.reshape((N_CORES,) + ps)[c]
            res.append(d)
        return res


_CACHE = {}


def _get_runner():
    if "runner" not in _CACHE:
        nc = build_kernel()
        _CACHE["nc"] = nc
        _CACHE["runner"] = _Runner(nc)
    return _CACHE["runner"]


def kernel(**inputs) -> np.ndarray:
    runner = _get_runner()
    in_maps = _prep_inputs(**inputs)
    args = runner.place(in_maps)
    outs = runner.run(args)
    res = runner.results(outs)
    return np.concatenate([res[c]["out"] for c in range(N_CORES)], axis=0)
